# revision 1
# baseline (speedup 1.0000x reference)
"""BigBird block Trainium2 kernel: 8-core SPMD.

Sharding: core c -> batch b = c//4, group g = c%4.
  - attention: heads 4g..4g+3 (C=256 qkv cols), tensor-parallel
  - FFN: hidden slice 1024g..1024(g+1), tensor-parallel
  - AllReduce (bf16) of attn-out partials within each 4-core group
  - FFN partials summed on host (gather/unshard step)

All compute uses feature-major ("transposed") layouts [feature, token] so
matmul contractions keep features on partitions.  LN1 is folded into the
QKV projections (per-token mu/rsig applied post-matmul); LN2 is explicit.
Softmax denominators ride along the attn@V matmul via a ones column
appended to each V tile (65-wide per-head stationary operand).
"""
import sys
from contextlib import ExitStack

sys.path.insert(0, "/opt/trn_rl_repo")
import numpy as np
import concourse.bacc as bacc
import concourse.mybir as mybir
from concourse import tile

F32 = mybir.dt.float32
F32R = mybir.dt.float32r
BF16 = mybir.dt.bfloat16
NPBF16 = mybir.dt.np(BF16)

B, T, D, H, HD = 2, 2048, 1024, 16, 64
C = 256          # qkv cols per core (4 heads)
FF = 1024        # ffn hidden per core
NCORES = 8
GROUPS = [[0, 1, 2, 3], [4, 5, 6, 7]]
DT8 = D // 128   # 8 d-tiles
TT16 = T // 128  # 16 token tiles
LN_EPS = 1e-5
AR_CHUNKS = ((0, 1024), (1024, 1536), (1536, 2048))

AF = mybir.ActivationFunctionType
OP = mybir.AluOpType


def r32(ap):
    return ap.bitcast(F32R)


def build_nc():
    nc = bacc.Bacc("TRN2", target_bir_lowering=False, debug=False,
                   num_devices=NCORES)
    dt = nc.dram_tensor
    xT = dt("xT", [D, T], F32R, kind="ExternalInput")
    maskT = dt("maskT", [T, T], BF16, kind="ExternalInput")
    wq = dt("wq", [D, C], F32R, kind="ExternalInput")
    wk = dt("wk", [D, C], F32R, kind="ExternalInput")
    wv = dt("wv", [D, C], F32R, kind="ExternalInput")
    wo = dt("wo", [C, D], F32R, kind="ExternalInput")
    w1 = dt("w1", [D, FF], F32R, kind="ExternalInput")
    w2 = dt("w2", [FF, D], BF16, kind="ExternalInput")
    wsq = dt("wsq", [128, 2], F32, kind="ExternalInput")   # colsum of wq
    wsk = dt("wsk", [128, 2], F32, kind="ExternalInput")
    wsv_bc = dt("wsv_bc", [128, C], F32, kind="ExternalInput")
    bq = dt("bq", [128, 2], F32, kind="ExternalInput")     # ln1_b @ Wq
    bk = dt("bk", [128, 2], F32, kind="ExternalInput")
    bv_bc = dt("bv_bc", [128, C], F32, kind="ExternalInput")
    bo_col = dt("bo_col", [128, DT8], F32, kind="ExternalInput")
    b1_col = dt("b1_col", [128, FF // 128], F32, kind="ExternalInput")
    wsum1 = dt("wsum1", [128, FF // 128], F32, kind="ExternalInput")

    x2T_out = dt("x2T_out", [D, T], F32, kind="ExternalOutput")
    ffT_out = dt("ffT_out", [D, T], BF16, kind="ExternalOutput")
    ar_in = [dt(f"ar_in{i}", [D, c1 - c0], BF16, kind="Internal")
             for i, (c0, c1) in enumerate(AR_CHUNKS)]
    ar_out = [dt(f"ar_out{i}", [D, c1 - c0], BF16, kind="Internal")
              for i, (c0, c1) in enumerate(AR_CHUNKS)]

    with ExitStack() as es:
        es.enter_context(nc.allow_low_precision(
            reason="fp32r SBUF tiles feed the PE; accumulation stays f32"))
        tc = es.enter_context(tile.TileContext(nc))

        def pool(name, bufs, space="SBUF"):
            return tc.tile_pool(name=name, bufs=bufs, space=space)

        pp = es.enter_context(pool("persist", 1))
        ones_sb = pp.tile([128, 128], F32R, name="ones_sb")
        nc.gpsimd.memset(ones_sb[:].bitcast(F32), 1.0)
        inv128 = pp.tile([128, 1], F32R, name="inv128")
        nc.gpsimd.memset(inv128[:].bitcast(F32), 1.0 / 128.0)

        # qT/kT/v live from phase 2 through phase 4
        s234 = es.enter_context(ExitStack())
        qsb = s234.enter_context(pool("qkv_sb", 1))

        # ================ phases 1+2: LN1 stats + QKV ====================
        with ExitStack() as s12:
            xres = s12.enter_context(pool("xres", 1))
            xts = []
            for d in range(DT8):
                t_ = xres.tile([128, T], F32R, tag=f"xt{d}", name=f"xt{d}")
                nc.sync.dma_start(t_[:], xT[d * 128:(d + 1) * 128, :])
                xts.append(t_)

            statsb = s12.enter_context(pool("statsb", 1))
            mu_bc = statsb.tile([128, T], F32, tag="mu", name="mu")
            nrsig_bc = statsb.tile([128, T], F32R, tag="nrsig", name="nrsig")
            murs_bc = statsb.tile([128, T], F32R, tag="murs", name="murs")
            wrk = statsb.tile([128, T], F32, tag="wrk", name="wrk")

            with pool("sqp", 2) as sqp, pool("statps", 1, "PSUM") as statps:
                sum_ps = [statps.tile([128, 512], F32, tag=f"sum{n}", name=f"sum{n}")
                          for n in range(4)]
                sq_ps = [statps.tile([128, 512], F32, tag=f"sq{n}", name=f"sq{n}")
                         for n in range(4)]
                for d in range(DT8):
                    sq = sqp.tile([128, T], F32R, tag="sq", name="sq")
                    nc.scalar.activation(sq[:], xts[d][:], AF.Square)
                    for n in range(4):
                        sl = slice(n * 512, (n + 1) * 512)
                        nc.tensor.matmul(sum_ps[n][:], r32(ones_sb[:]),
                                         r32(xts[d][:, sl]),
                                         start=(d == 0), stop=(d == DT8 - 1),
                                         skip_group_check=True)
                        nc.tensor.matmul(sq_ps[n][:], r32(ones_sb[:]),
                                         r32(sq[:, sl]),
                                         start=(d == 0), stop=(d == DT8 - 1),
                                         skip_group_check=True)
                for n in range(4):
                    sl = slice(n * 512, (n + 1) * 512)
                    nc.vector.tensor_scalar_mul(mu_bc[:, sl], sum_ps[n][:],
                                                1.0 / D)
                    nc.vector.tensor_scalar_mul(wrk[:, sl], sq_ps[n][:],
                                                1.0 / D)
            # var = E[x^2] - mu^2 + eps; rsig = 1/sqrt(var)
            nc.vector.tensor_tensor(murs_bc[:], mu_bc[:], mu_bc[:], OP.mult)
            nc.vector.tensor_sub(wrk[:], wrk[:], murs_bc[:])
            nc.vector.tensor_scalar_add(wrk[:], wrk[:], LN_EPS)
            nc.scalar.activation(wrk[:], wrk[:], AF.Sqrt)
            nc.vector.reciprocal(murs_bc[:], wrk[:])          # rsig (temp)
            nc.vector.tensor_scalar_mul(nrsig_bc[:], murs_bc[:], -1.0)
            nc.vector.tensor_tensor(murs_bc[:], mu_bc[:], murs_bc[:],
                                    OP.mult)                  # mu*rsig

            # per-token scalar columns (for the v path)
            rsig_col, murs_col = [], []
            with pool("colps", 2, "PSUM") as cps:
                for tt in range(TT16):
                    sl = slice(tt * 128, (tt + 1) * 128)
                    pr = cps.tile([128, 1], F32, tag="pr", name="pr")
                    nc.tensor.matmul(pr[:], nrsig_bc[:, sl].bitcast(F32),
                                     inv128[:].bitcast(F32),
                                     start=True, stop=True,
                                     skip_group_check=True)
                    rc = statsb.tile([128, 1], F32, tag=f"rc{tt}", name=f"rc{tt}")
                    nc.vector.tensor_scalar_mul(rc[:], pr[:], -1.0)
                    rsig_col.append(rc)
                    pm = cps.tile([128, 1], F32, tag="pm", name="pm")
                    nc.tensor.matmul(pm[:], murs_bc[:, sl].bitcast(F32),
                                     inv128[:].bitcast(F32),
                                     start=True, stop=True,
                                     skip_group_check=True)
                    mc = statsb.tile([128, 1], F32, tag=f"mc{tt}", name=f"mc{tt}")
                    nc.vector.tensor_copy(mc[:], pm[:])
                    murs_col.append(mc)

            # ---- QKV ----
            with pool("wqkv", 1) as wp, pool("qkps", 4, "PSUM") as qkps, \
                 pool("qtmp", 2) as qtmp:
                wq_sb, wk_sb, wv_sb = [], [], []
                for d in range(DT8):
                    for nm, dr, lst in (("wq", wq, wq_sb), ("wk", wk, wk_sb),
                                        ("wv", wv, wv_sb)):
                        w_ = wp.tile([128, C], F32R, tag=f"{nm}{d}", name=f"{nm}{d}")
                        nc.sync.dma_start(w_[:],
                                          dr[d * 128:(d + 1) * 128, :])
                        lst.append(w_)
                scal = {}
                for nm, dr in (("wsq", wsq), ("wsk", wsk), ("bq", bq),
                               ("bk", bk)):
                    s_ = wp.tile([128, 2], F32, tag=nm, name=nm)
                    nc.sync.dma_start(s_[:], dr[:])
                    scal[nm] = s_
                wsv_sb = wp.tile([128, C], F32, tag="wsv", name="wsv")
                nc.sync.dma_start(wsv_sb[:], wsv_bc[:])
                bv_sb = wp.tile([128, C], F32, tag="bv", name="bv")
                nc.sync.dma_start(bv_sb[:], bv_bc[:])

                qT, kT = [], []
                for zname, wz, lst, ws_key, b_key in (
                        ("q", wq_sb, qT, "wsq", "bq"),
                        ("k", wk_sb, kT, "wsk", "bk")):
                    for m in range(2):
                        zt = qsb.tile([128, T], F32R, tag=f"{zname}T{m}", name=f"{zname}T{m}")
                        msl = slice(m * 128, (m + 1) * 128)
                        for n in range(4):
                            nsl = slice(n * 512, (n + 1) * 512)
                            zp = qkps.tile([128, 512], F32, tag="zp", name="zp")
                            for d in range(DT8):
                                nc.tensor.matmul(
                                    zp[:], r32(wz[d][:, msl]),
                                    r32(xts[d][:, nsl]),
                                    start=(d == 0), stop=(d == DT8 - 1),
                                    skip_group_check=True)
                            # (mu*wsz - raw); then z = that*(-rsig) + b
                            nc.vector.scalar_tensor_tensor(
                                zt[:, nsl], mu_bc[:, nsl],
                                scal[ws_key][:, m:m + 1], zp[:],
                                OP.mult, OP.subtract)
                        nc.vector.tensor_tensor(zt[:], zt[:], nrsig_bc[:],
                                                OP.mult)
                        nc.vector.tensor_scalar_add(zt[:], zt[:],
                                                    scal[b_key][:, m:m + 1])
                        lst.append(zt)

                # v natural [t-part, c-free] bf16, 65-stride + ones column
                v_sb = []
                for tt in range(TT16):
                    vt = qsb.tile([128, 4 * 65], BF16, tag=f"v{tt}", name=f"v{tt}")
                    v3 = vt[:].rearrange("p (h c) -> p h c", h=4)
                    nc.gpsimd.memset(v3[:, :, 64:65], 1.0)
                    vp = qkps.tile([128, C], F32, tag="vp", name="vp")
                    tsl = slice(tt * 128, (tt + 1) * 128)
                    for d in range(DT8):
                        nc.tensor.matmul(vp[:], r32(xts[d][:, tsl]),
                                         r32(wv_sb[d][:]),
                                         start=(d == 0), stop=(d == DT8 - 1),
                                         skip_group_check=True)
                    tmp2 = qtmp.tile([128, C], F32, tag="tmp2", name="tmp2")
                    nc.vector.tensor_scalar(tmp2[:], wsv_sb[:],
                                            murs_col[tt][:], None, OP.mult)
                    nc.vector.tensor_sub(tmp2[:], bv_sb[:], tmp2[:])
                    vp3 = vp[:].rearrange("p (h c) -> p h c", h=4)
                    t23 = tmp2[:].rearrange("p (h c) -> p h c", h=4)
                    nc.vector.scalar_tensor_tensor(
                        v3[:, :, 0:64], vp3[:, :, :], rsig_col[tt][:],
                        t23[:, :, :], OP.mult, OP.add)
                    v_sb.append(vt)
        # s12 closed: xts + stats freed; qT/kT/v_sb persist in qsb.

        # ================ phase 3+4: attention, Wo, chunked AllReduce ====
        asb = s234.enter_context(pool("att_sb", 1))
        attnT = [asb.tile([128, T], F32R, tag=f"aT{m}", name=f"aT{m}") for m in range(2)]
        den = [asb.tile([1, T], F32, tag=f"den{h}", name=f"den{h}")
               for h in range(4)]
        rden = [asb.tile([1, T], F32R, tag=f"rden{h}", name=f"rden{h}")
                for h in range(4)]
        wo_sb = []
        for cc in range(2):
            w_ = asb.tile([128, D], F32R, tag=f"wo{cc}", name=f"wo{cc}")
            nc.sync.dma_start(w_[:], wo[cc * 128:(cc + 1) * 128, :])
            wo_sb.append(w_)
        with pool("mskp", 1) as mskp, pool("ptp", 4) as ptp, \
             pool("sps", 2, "PSUM") as spsp, \
             pool("avps", 1, "PSUM") as avps, \
             pool("dnps", 1, "PSUM") as dnps, \
             pool("arp", 2) as arp:
            for j in range(4):
                n_kt = 4 * j + 4
                qsl = slice(j * 512, (j + 1) * 512)
                mts = {}
                for kt in range(n_kt):
                    ksl = slice(kt * 128, (kt + 1) * 128)
                    mt = mskp.tile([128, 512], BF16, tag=f"mt{kt}",
                                   name=f"mt{kt}")
                    nc.sync.dma_start(mt[:], maskT[ksl, qsl])
                    mts[kt] = mt
                for hp in range(2):
                    avA = avps.tile([65, 512], F32, tag="avA", name="avA")
                    avB = avps.tile([65, 512], F32, tag="avB", name="avB")
                    for kt in range(n_kt):
                        ksl = slice(kt * 128, (kt + 1) * 128)
                        sps = spsp.tile([128, 1024], F32, tag="sps", name="sps")
                        nc.tensor.matmul(
                            sps[:, 0:512], r32(kT[hp][0:64, ksl]),
                            r32(qT[hp][0:64, qsl]), start=True, stop=True,
                            tile_position=(0, 0), skip_group_check=True)
                        nc.tensor.matmul(
                            sps[:, 512:1024], r32(kT[hp][64:128, ksl]),
                            r32(qT[hp][64:128, qsl]), start=True, stop=True,
                            tile_position=(64, 0), skip_group_check=True)
                        pt = ptp.tile([128, 1024], BF16, tag="pt", name="pt")
                        nc.scalar.activation(pt[:], sps[:], AF.Exp,
                                             scale=0.125)
                        nc.vector.tensor_mul(pt[:, 0:512], pt[:, 0:512],
                                             mts[kt][:])
                        nc.vector.tensor_mul(pt[:, 512:1024],
                                             pt[:, 512:1024], mts[kt][:])
                        vv = v_sb[kt][:].rearrange("p (h c) -> p h c", h=4)
                        nc.tensor.matmul(
                            avA[:], vv[:, 2 * hp, :], pt[:, 0:512],
                            start=(kt == 0), stop=(kt == n_kt - 1),
                            skip_group_check=True)
                        nc.tensor.matmul(
                            avB[:], vv[:, 2 * hp + 1, :], pt[:, 512:1024],
                            start=(kt == 0), stop=(kt == n_kt - 1),
                            skip_group_check=True)
                    nc.scalar.copy(attnT[hp][0:64, qsl], avA[0:64, :])
                    nc.scalar.copy(attnT[hp][64:128, qsl], avB[0:64, :])
                    nc.vector.tensor_copy(den[2 * hp][:, qsl], avA[64:65, :])
                    nc.vector.tensor_copy(den[2 * hp + 1][:, qsl],
                                          avB[64:65, :])
                # normalize chunk j and project through Wo, then AR chunk j
                for h in range(4):
                    nc.vector.reciprocal(rden[h][:, qsl], den[h][:, qsl])
                for m in range(2):
                    dp = dnps.tile([128, 512], F32, tag="dp", name="dp")
                    nc.tensor.matmul(dp[0:64, :],
                                     ones_sb[0:1, 0:64].bitcast(F32),
                                     rden[2 * m][:, qsl].bitcast(F32),
                                     start=True, stop=True,
                                     skip_group_check=True)
                    nc.tensor.matmul(dp[64:128, :],
                                     ones_sb[0:1, 64:128].bitcast(F32),
                                     rden[2 * m + 1][:, qsl].bitcast(F32),
                                     start=True, stop=True,
                                     skip_group_check=True)
                    nc.vector.tensor_mul(attnT[m][:, qsl], attnT[m][:, qsl],
                                         dp[:])
                for o in range(DT8):
                    osl = slice(o * 128, (o + 1) * 128)
                    wps = dnps.tile([128, 512], F32, tag="wps", name="wps")
                    for cc in range(2):
                        nc.tensor.matmul(
                            wps[:], r32(wo_sb[cc][:, osl]),
                            r32(attnT[cc][:, qsl]),
                            start=(cc == 0), stop=(cc == 1),
                            skip_group_check=True)
                    ao = arp.tile([128, 512], BF16, tag="ao", name="ao")
                    nc.scalar.copy(ao[:], wps[:])
                    ci = next(i for i, (c0, c1) in enumerate(AR_CHUNKS)
                              if c0 <= j * 512 < c1)
                    c0 = AR_CHUNKS[ci][0]
                    nc.sync.dma_start(
                        ar_in[ci][osl, j * 512 - c0:(j + 1) * 512 - c0],
                        ao[:])
                if any(c1 == (j + 1) * 512 for c0, c1 in AR_CHUNKS):
                    ci = next(i for i, (c0, c1) in enumerate(AR_CHUNKS)
                              if c1 == (j + 1) * 512)
                    nc.gpsimd.collective_compute(
                        "AllReduce", mybir.AluOpType.add,
                        replica_groups=GROUPS,
                        ins=[ar_in[ci][:]], outs=[ar_out[ci][:]])
        s234.close()  # free qT/kT/v/attnT SBUF before phases 5-6

        # ================ phases 5+6: per-token-chunk x2+LN2+FFN =========
        with pool("x2p", 2) as x2p, pool("ln2sb", 2) as ln2sb, \
             pool("sqp2", 2) as sqp2, pool("arl", 2) as arl, \
             pool("lnps", 2, "PSUM") as lnps, \
             pool("ffps", 2, "PSUM") as ffps, \
             pool("wfp", 1) as wfp, pool("a1p", 1) as a1p, \
             pool("ffo", 2) as ffop:
            bo_sb = ln2sb.tile([128, DT8], F32, tag="bo", name="bo", bufs=1)
            nc.sync.dma_start(bo_sb[:], bo_col[:])
            b1_sb = wfp.tile([128, FF // 128], F32, tag="b1", name="b1")
            nc.sync.dma_start(b1_sb[:], b1_col[:])
            ws1_sb = wfp.tile([128, FF // 128], F32, tag="ws1", name="ws1")
            nc.sync.dma_start(ws1_sb[:], wsum1[:])
            w1_sb, w2_sb = [], []
            for d in range(DT8):
                dsl = slice(d * 128, (d + 1) * 128)
                w1t = wfp.tile([128, FF], F32R, tag=f"w1_{d}", name=f"w1_{d}")
                nc.sync.dma_start(w1t[:], w1[dsl, :])
                w1_sb.append(w1t)
                w2t = wfp.tile([128, D], BF16, tag=f"w2_{d}", name=f"w2_{d}")
                nc.sync.dma_start(w2t[:], w2[dsl, :])
                w2_sb.append(w2t)
            a1 = [a1p.tile([128, T], BF16, tag=f"a1_{hm}", name=f"a1_{hm}")
                  for hm in range(FF // 128)]
            for j in range(4):
                jsl = slice(j * 512, (j + 1) * 512)
                x2 = []
                for d in range(DT8):
                    dsl = slice(d * 128, (d + 1) * 128)
                    xt2 = x2p.tile([128, 512], F32R, tag=f"x2_{d}",
                                   name=f"x2_{d}")
                    xr = arl.tile([128, 512], F32, tag="xr", name="xr")
                    nc.sync.dma_start(xr[:], xT[dsl, jsl].bitcast(F32))
                    ar_t = arl.tile([128, 512], BF16, tag="art", name="art")
                    ci = next(i for i, (c0, c1) in enumerate(AR_CHUNKS)
                              if c0 <= j * 512 < c1)
                    c0 = AR_CHUNKS[ci][0]
                    nc.sync.dma_start(
                        ar_t[:],
                        ar_out[ci][dsl, j * 512 - c0:(j + 1) * 512 - c0])
                    nc.vector.scalar_tensor_tensor(
                        xt2[:], ar_t[:], bo_sb[:, d:d + 1], xr[:],
                        OP.add, OP.add)
                    nc.sync.dma_start(x2T_out[dsl, jsl], xt2[:].bitcast(F32))
                    x2.append(xt2)
                # LN2 stats for chunk j
                sum_ps = lnps.tile([128, 512], F32, tag="s2", name="s2")
                sq_ps = lnps.tile([128, 512], F32, tag="q2", name="q2")
                for d in range(DT8):
                    sq = sqp2.tile([128, 512], F32R, tag="sq2", name="sq2")
                    nc.scalar.activation(sq[:], x2[d][:], AF.Square)
                    nc.tensor.matmul(sum_ps[:], r32(ones_sb[:]),
                                     r32(x2[d][:]),
                                     start=(d == 0), stop=(d == DT8 - 1),
                                     skip_group_check=True)
                    nc.tensor.matmul(sq_ps[:], r32(ones_sb[:]), r32(sq[:]),
                                     start=(d == 0), stop=(d == DT8 - 1),
                                     skip_group_check=True)
                mu2 = ln2sb.tile([128, 512], F32, tag="mu2", name="mu2")
                nrsig2 = ln2sb.tile([128, 512], F32, tag="nrsig2",
                                    name="nrsig2")
                wrk2 = ln2sb.tile([128, 512], F32, tag="wrk2", name="wrk2")
                nc.vector.tensor_scalar_mul(mu2[:], sum_ps[:], 1.0 / D)
                nc.vector.tensor_scalar_mul(wrk2[:], sq_ps[:], 1.0 / D)
                nc.vector.tensor_tensor(nrsig2[:], mu2[:], mu2[:], OP.mult)
                nc.vector.tensor_sub(wrk2[:], wrk2[:], nrsig2[:])
                nc.vector.tensor_scalar_add(wrk2[:], wrk2[:], LN_EPS)
                nc.scalar.activation(wrk2[:], wrk2[:], AF.Sqrt)
                nc.vector.reciprocal(nrsig2[:], wrk2[:])
                murs2 = ln2sb.tile([128, 512], F32, tag="murs2",
                                   name="murs2")
                nc.vector.tensor_tensor(murs2[:], mu2[:], nrsig2[:], OP.mult)
                # x2s = x2 * rsig2 once; correction folded via stt + gelu
                # scale=-1:  a1n = a1s_raw - wsum1*murs2
                #            gelu(-( (murs2*wsum1) - a1s_raw ))
                x2s = []
                for d in range(DT8):
                    xs_ = sqp2.tile([128, 512], F32R, tag=f"x2s_{d}",
                                    name=f"x2s_{d}", bufs=2)
                    nc.vector.tensor_tensor(xs_[:], x2[d][:], nrsig2[:],
                                            OP.mult)
                    x2s.append(xs_)
                for hm in range(FF // 128):
                    hsl = slice(hm * 128, (hm + 1) * 128)
                    ap_ = ffps.tile([128, 512], F32, tag="a1ps", name="a1ps")
                    for d in range(DT8):
                        nc.tensor.matmul(ap_[:], w1_sb[d][:, hsl],
                                         x2s[d][:],
                                         start=(d == 0), stop=(d == DT8 - 1),
                                         skip_group_check=True)
                    a1n = sqp2.tile([128, 512], F32, tag="a1n", name="a1n")
                    nc.vector.scalar_tensor_tensor(
                        a1n[:], murs2[:], ws1_sb[:, hm:hm + 1], ap_[:],
                        OP.mult, OP.subtract)
                    nc.scalar.activation(a1[hm][:, jsl], a1n[:], AF.Gelu,
                                         bias=b1_sb[:, hm:hm + 1], scale=-1.0)
                for om in range(DT8):
                    osl = slice(om * 128, (om + 1) * 128)
                    fo = ffop.tile([128, 512], BF16, tag="fo", name="fo")
                    fp_ = ffps.tile([128, 512], F32, tag="ffps", name="ffps")
                    for hm in range(FF // 128):
                        nc.tensor.matmul(fp_[:], w2_sb[hm][:, osl],
                                         a1[hm][:, jsl],
                                         start=(hm == 0),
                                         stop=(hm == FF // 128 - 1),
                                         skip_group_check=True)
                    nc.scalar.copy(fo[:], fp_[:])
                    nc.sync.dma_start(ffT_out[osl, jsl], fo[:])
    nc.compile()
    return nc


def host_prep(inputs):
    """Build per-core input maps from the full problem inputs."""
    x = np.asarray(inputs["x"], np.float32)
    mask = np.asarray(inputs["mask"])
    ln1_g = np.asarray(inputs["ln1_g"], np.float32)
    ln1_b = np.asarray(inputs["ln1_b"], np.float32)
    ln2_g = np.asarray(inputs["ln2_g"], np.float32)
    ln2_b = np.asarray(inputs["ln2_b"], np.float32)
    Wq = np.asarray(inputs["Wq"], np.float32)
    Wk = np.asarray(inputs["Wk"], np.float32)
    Wv = np.asarray(inputs["Wv"], np.float32)
    Wo = np.asarray(inputs["Wo"], np.float32)
    bo = np.asarray(inputs["bo"], np.float32)
    W1 = np.asarray(inputs["W1"], np.float32)
    b1 = np.asarray(inputs["b1"], np.float32)
    W2 = np.asarray(inputs["W2"], np.float32)
    b2 = np.asarray(inputs["b2"], np.float32)

    maskT = np.ascontiguousarray(mask.T).astype(np.float32).astype(NPBF16)
    Wq_f = ln1_g[:, None] * Wq
    Wk_f = ln1_g[:, None] * Wk
    Wv_f = ln1_g[:, None] * Wv
    W1_f = ln2_g[:, None] * W1
    in_maps = []
    for c in range(NCORES):
        b, g = divmod(c, 4)
        cs = slice(g * C, (g + 1) * C)
        fs = slice(g * FF, (g + 1) * FF)
        wq_s = np.ascontiguousarray(Wq_f[:, cs])
        wk_s = np.ascontiguousarray(Wk_f[:, cs])
        wv_s = np.ascontiguousarray(Wv_f[:, cs])
        m = {
            "xT": np.ascontiguousarray(x[b].T),
            "maskT": maskT,
            "wq": wq_s, "wk": wk_s, "wv": wv_s,
            "wo": np.ascontiguousarray(Wo[cs, :]),
            "w1": np.ascontiguousarray(W1_f[:, fs]),
            "w2": np.ascontiguousarray(W2[fs, :]).astype(NPBF16),
            "wsq": wq_s.sum(0).reshape(2, 128).T.copy(),
            "wsk": wk_s.sum(0).reshape(2, 128).T.copy(),
            "wsv_bc": np.broadcast_to(wv_s.sum(0), (128, C)).copy(),
            "bq": (ln1_b @ Wq[:, cs]).reshape(2, 128).T.copy(),
            "bk": (ln1_b @ Wk[:, cs]).reshape(2, 128).T.copy(),
            "bv_bc": np.broadcast_to(ln1_b @ Wv[:, cs], (128, C)).copy(),
            "bo_col": bo.reshape(DT8, 128).T.copy(),
            "b1_col": (ln2_b @ W1_f[:, fs] + b1[fs]).reshape(FF // 128, 128).T.copy(),
            "wsum1": W1_f[:, fs].sum(0).reshape(FF // 128, 128).T.copy(),
        }
        in_maps.append(m)
    return in_maps, b2


def host_assemble(out_maps, b2):
    out = np.empty((B, T, D), np.float32)
    for b in range(B):
        x2T = out_maps[4 * b]["x2T_out"].astype(np.float32)
        ff = sum(out_maps[4 * b + g]["ffT_out"].astype(np.float32)
                 for g in range(4))
        out[b] = (x2T + ff + b2[:, None]).T
    return out


# ======================================================================
# Harness entry point
# ======================================================================
_NC_CACHE = {}


def _get_nc():
    if "nc" not in _NC_CACHE:
        _NC_CACHE["nc"] = build_nc()
    return _NC_CACHE["nc"]


def kernel(**inputs):
    """Full-input / full-output BigBird block on 8 NeuronCores."""
    from concourse.bass_utils import run_bass_kernel_spmd
    nc = _get_nc()
    in_maps, b2 = host_prep(inputs)
    res = run_bass_kernel_spmd(nc, in_maps, list(range(NCORES)))
    return host_assemble(res.results, b2)



# revision 28
# speedup vs baseline: 5.1108x; 5.1108x over previous
"""BigBird block Trainium2 kernel: 8-core SPMD.

Sharding: core c -> batch b = c//4, group g = c%4.
  - attention: heads 4g..4g+3 (C=256 qkv cols), tensor-parallel
  - ReduceScatter (bf16) of attn-out partials within each 4-core group:
    core c receives the summed attn-out for ITS 512-token slice
  - FFN: token-parallel (512 tokens per core, full 4096 hidden, streamed
    bf16 weights); final out = x2 + ff + b2 computed fully in-kernel

All compute uses feature-major ("transposed") layouts [feature, token] so
matmul contractions keep features on partitions.  LN1 is folded into the
QKV projections (per-token mu/rsig applied post-matmul); LN2 is explicit.
Softmax denominators ride along the attn@V matmul via a ones column
appended to each V tile (65-wide per-head stationary operand).
"""
import sys
from contextlib import ExitStack

sys.path.insert(0, "/opt/trn_rl_repo")
import numpy as np
import concourse.bacc as bacc
import concourse.mybir as mybir
from concourse import tile

F32 = mybir.dt.float32
F32R = mybir.dt.float32r
BF16 = mybir.dt.bfloat16
NPBF16 = mybir.dt.np(BF16)

B, T, D, H, HD = 2, 2048, 1024, 16, 64
C = 256          # qkv cols per core (4 heads)
FF = 4096        # full ffn hidden (token-parallel ffn)
TC = 512         # tokens per core in the ffn/output phase
NCORES = 8
GROUPS = [[0, 1, 2, 3], [4, 5, 6, 7]]
DT8 = D // 128   # 8 d-tiles
TT16 = T // 128  # 16 token tiles
HM32 = FF // 128  # 32 hidden tiles
LN_EPS = 1e-5

AF = mybir.ActivationFunctionType
OP = mybir.AluOpType


def r32(ap):
    return ap.bitcast(F32R)


def build_nc():
    nc = bacc.Bacc("TRN2", target_bir_lowering=False, debug=False,
                   num_devices=NCORES)
    dt = nc.dram_tensor
    xT = dt("xT", [D, T], F32R, kind="ExternalInput")
    maskT = dt("maskT", [T, T], BF16, kind="ExternalInput")
    x_myT = dt("x_myT", [D, TC], F32, kind="ExternalInput")
    wq = dt("wq", [D, C], F32R, kind="ExternalInput")
    wk = dt("wk", [D, C], F32R, kind="ExternalInput")
    wv = dt("wv", [D, C], F32R, kind="ExternalInput")
    wo = dt("wo", [C, D], F32R, kind="ExternalInput")
    w1r = dt("w1r", [128, HM32 * DT8 * 128], BF16, kind="ExternalInput")
    w2r = dt("w2r", [128, DT8 * HM32 * 128], BF16, kind="ExternalInput")
    wsq = dt("wsq", [128, 2], F32, kind="ExternalInput")   # colsum of wq
    wsk = dt("wsk", [128, 2], F32, kind="ExternalInput")
    wsv_bc = dt("wsv_bc", [128, C], F32, kind="ExternalInput")
    bq = dt("bq", [128, 2], F32, kind="ExternalInput")     # ln1_b @ Wq
    bk = dt("bk", [128, 2], F32, kind="ExternalInput")
    bv_bc = dt("bv_bc", [128, C], F32, kind="ExternalInput")
    bo_col = dt("bo_col", [128, DT8], F32, kind="ExternalInput")
    b1_col = dt("b1_col", [128, HM32], F32, kind="ExternalInput")
    wsum1 = dt("wsum1", [128, HM32], F32, kind="ExternalInput")
    b2_col = dt("b2_col", [128, DT8], F32, kind="ExternalInput")

    outT = dt("outT", [D, TC], F32, kind="ExternalOutput")
    rs_in = dt("rs_in", [4 * D, TC], BF16, kind="Internal")
    rs_out = dt("rs_out", [D, TC], BF16, kind="Internal")

    with ExitStack() as es:
        es.enter_context(nc.allow_low_precision(
            reason="fp32r SBUF tiles feed the PE; accumulation stays f32"))
        tc = es.enter_context(tile.TileContext(nc))

        def pool(name, bufs, space="SBUF"):
            return tc.tile_pool(name=name, bufs=bufs, space=space)

        pp = es.enter_context(pool("persist", 1))
        ones_sb = pp.tile([128, 128], F32R, name="ones_sb")
        nc.gpsimd.memset(ones_sb[:].bitcast(F32), 1.0)
        inv128 = pp.tile([128, 1], F32R, name="inv128")
        nc.gpsimd.memset(inv128[:].bitcast(F32), 1.0 / 128.0)

        # FFN bias pool (tiny) lives until module end.
        wfp = es.enter_context(pool("wfp", 1))

        # qT/kT/v live from phase 2 through phase 4
        s234 = es.enter_context(ExitStack())
        qsb = s234.enter_context(pool("qkv_sb", 1))

        # ================ phases 1+2: LN1 stats + QKV ====================
        with ExitStack() as s12:
            xres = s12.enter_context(pool("xres", 1))
            xts = []
            for d in range(DT8):
                t_ = xres.tile([128, T], F32R, tag=f"xt{d}", name=f"xt{d}")
                eng = nc.sync if d % 2 == 0 else nc.scalar
                eng.dma_start(t_[:], xT[d * 128:(d + 1) * 128, :])
                xts.append(t_)

            statsb = s12.enter_context(pool("statsb", 1))
            mu_bc = statsb.tile([128, T], F32, tag="mu", name="mu")
            nrsig_bc = statsb.tile([128, T], F32R, tag="nrsig", name="nrsig")
            murs_bc = statsb.tile([128, T], F32R, tag="murs", name="murs")
            wrk = statsb.tile([128, T], F32, tag="wrk", name="wrk")

            with pool("sqp", 2) as sqp, pool("statps", 1, "PSUM") as statps:
                sum_ps = [statps.tile([128, 512], F32, tag=f"sum{n}", name=f"sum{n}")
                          for n in range(4)]
                sq_ps = [statps.tile([128, 512], F32, tag=f"sq{n}", name=f"sq{n}")
                         for n in range(4)]
                for d in range(DT8):
                    sq = sqp.tile([128, T], F32R, tag="sq", name="sq")
                    nc.scalar.activation(sq[:], xts[d][:], AF.Square)
                    for n in range(4):
                        sl = slice(n * 512, (n + 1) * 512)
                        nc.tensor.matmul(sum_ps[n][:], r32(ones_sb[:]),
                                         r32(xts[d][:, sl]),
                                         start=(d == 0), stop=(d == DT8 - 1),
                                         skip_group_check=True)
                        nc.tensor.matmul(sq_ps[n][:], r32(ones_sb[:]),
                                         r32(sq[:, sl]),
                                         start=(d == 0), stop=(d == DT8 - 1),
                                         skip_group_check=True)
                for n in range(4):
                    sl = slice(n * 512, (n + 1) * 512)
                    nc.vector.tensor_scalar_mul(mu_bc[:, sl], sum_ps[n][:],
                                                1.0 / D)
                    nc.vector.tensor_scalar_mul(wrk[:, sl], sq_ps[n][:],
                                                1.0 / D)
            # var = E[x^2] - mu^2 + eps; rsig = 1/sqrt(var)
            nc.vector.tensor_tensor(murs_bc[:], mu_bc[:], mu_bc[:], OP.mult)
            nc.vector.tensor_sub(wrk[:], wrk[:], murs_bc[:])
            nc.vector.tensor_scalar_add(wrk[:], wrk[:], LN_EPS)
            nc.scalar.activation(wrk[:], wrk[:], AF.Sqrt)
            nc.vector.reciprocal(murs_bc[:], wrk[:])          # rsig (temp)
            nc.vector.tensor_scalar_mul(nrsig_bc[:], murs_bc[:], -1.0)
            nc.vector.tensor_tensor(murs_bc[:], mu_bc[:], murs_bc[:],
                                    OP.mult)                  # mu*rsig

            # per-token scalar columns (for the v path)
            rsig_col, murs_col = [], []
            with pool("colps", 2, "PSUM") as cps:
                for tt in range(TT16):
                    sl = slice(tt * 128, (tt + 1) * 128)
                    pr = cps.tile([128, 1], F32, tag="pr", name="pr")
                    nc.tensor.matmul(pr[:], nrsig_bc[:, sl].bitcast(F32),
                                     inv128[:].bitcast(F32),
                                     start=True, stop=True,
                                     skip_group_check=True)
                    rc = statsb.tile([128, 1], F32, tag=f"rc{tt}", name=f"rc{tt}")
                    nc.vector.tensor_scalar_mul(rc[:], pr[:], -1.0)
                    rsig_col.append(rc)
                    pm = cps.tile([128, 1], F32, tag="pm", name="pm")
                    nc.tensor.matmul(pm[:], murs_bc[:, sl].bitcast(F32),
                                     inv128[:].bitcast(F32),
                                     start=True, stop=True,
                                     skip_group_check=True)
                    mc = statsb.tile([128, 1], F32, tag=f"mc{tt}", name=f"mc{tt}")
                    nc.vector.tensor_copy(mc[:], pm[:])
                    murs_col.append(mc)

            # ---- QKV ----
            with pool("wqkv", 1) as wp, pool("qkps", 4, "PSUM") as qkps, \
                 pool("qtmp", 2) as qtmp:
                wq_sb, wk_sb, wv_sb = [], [], []
                for d in range(DT8):
                    for nm, dr, lst in (("wq", wq, wq_sb), ("wk", wk, wk_sb),
                                        ("wv", wv, wv_sb)):
                        w_ = wp.tile([128, C], F32R, tag=f"{nm}{d}", name=f"{nm}{d}")
                        eng = nc.sync if d % 2 == 0 else nc.scalar
                        eng.dma_start(w_[:], dr[d * 128:(d + 1) * 128, :])
                        lst.append(w_)
                scal = {}
                for nm, dr in (("wsq", wsq), ("wsk", wsk), ("bq", bq),
                               ("bk", bk)):
                    s_ = wp.tile([128, 2], F32, tag=nm, name=nm)
                    nc.sync.dma_start(s_[:], dr[:])
                    scal[nm] = s_
                wsv_sb = wp.tile([128, C], F32, tag="wsv", name="wsv")
                nc.sync.dma_start(wsv_sb[:], wsv_bc[:])
                bv_sb = wp.tile([128, C], F32, tag="bv", name="bv")
                nc.sync.dma_start(bv_sb[:], bv_bc[:])

                qT, kT = [], []
                for zname, wz, lst, ws_key, b_key in (
                        ("q", wq_sb, qT, "wsq", "bq"),
                        ("k", wk_sb, kT, "wsk", "bk")):
                    for m in range(2):
                        zt = qsb.tile([128, T], F32R, tag=f"{zname}T{m}", name=f"{zname}T{m}")
                        msl = slice(m * 128, (m + 1) * 128)
                        for n in range(4):
                            nsl = slice(n * 512, (n + 1) * 512)
                            zp = qkps.tile([128, 512], F32, tag="zp", name="zp")
                            for d in range(DT8):
                                nc.tensor.matmul(
                                    zp[:], r32(wz[d][:, msl]),
                                    r32(xts[d][:, nsl]),
                                    start=(d == 0), stop=(d == DT8 - 1),
                                    skip_group_check=True)
                            # (mu*wsz - raw); then z = that*(-rsig) + b
                            nc.vector.scalar_tensor_tensor(
                                zt[:, nsl], mu_bc[:, nsl],
                                scal[ws_key][:, m:m + 1], zp[:],
                                OP.mult, OP.subtract)
                        nc.vector.tensor_tensor(zt[:], zt[:], nrsig_bc[:],
                                                OP.mult)
                        nc.vector.tensor_scalar_add(zt[:], zt[:],
                                                    scal[b_key][:, m:m + 1])
                        lst.append(zt)

                # v natural [t-part, c-free] bf16, 65-stride + ones column
                v_sb = []
                for tt in range(TT16):
                    vt = qsb.tile([128, 4 * 65], BF16, tag=f"v{tt}", name=f"v{tt}")
                    v3 = vt[:].rearrange("p (h c) -> p h c", h=4)
                    nc.gpsimd.memset(v3[:, :, 64:65], 1.0)
                    vp = qkps.tile([128, C], F32, tag="vp", name="vp")
                    tsl = slice(tt * 128, (tt + 1) * 128)
                    for d in range(DT8):
                        nc.tensor.matmul(vp[:], r32(xts[d][:, tsl]),
                                         r32(wv_sb[d][:]),
                                         start=(d == 0), stop=(d == DT8 - 1),
                                         skip_group_check=True)
                    tmp2 = qtmp.tile([128, C], F32, tag="tmp2", name="tmp2")
                    nc.vector.tensor_scalar(tmp2[:], wsv_sb[:],
                                            murs_col[tt][:], None, OP.mult)
                    nc.vector.tensor_sub(tmp2[:], bv_sb[:], tmp2[:])
                    vp3 = vp[:].rearrange("p (h c) -> p h c", h=4)
                    t23 = tmp2[:].rearrange("p (h c) -> p h c", h=4)
                    nc.vector.scalar_tensor_tensor(
                        v3[:, :, 0:64], vp3[:, :, :], rsig_col[tt][:],
                        t23[:, :, :], OP.mult, OP.add)
                    v_sb.append(vt)
        # s12 closed: xts + stats freed; qT/kT/v_sb persist in qsb.

        # ================ phase 3+4: attention, Wo, ReduceScatter ========
        asb = s234.enter_context(pool("att_sb", 2))
        wo_sb = []
        for cc in range(2):
            w_ = asb.tile([128, D], F32R, tag=f"wo{cc}", name=f"wo{cc}")
            nc.sync.dma_start(w_[:], wo[cc * 128:(cc + 1) * 128, :])
            wo_sb.append(w_)
        # FFN biases: tiny loads, issued before the attention j-loop.
        bo_sb = wfp.tile([128, DT8], F32, tag="bo", name="bo")
        nc.sync.dma_start(bo_sb[:], bo_col[:])
        b2_sb = wfp.tile([128, DT8], F32, tag="b2", name="b2")
        nc.sync.dma_start(b2_sb[:], b2_col[:])
        b1_sb = wfp.tile([128, HM32], F32, tag="b1", name="b1")
        nc.sync.dma_start(b1_sb[:], b1_col[:])
        ws1_sb = wfp.tile([128, HM32], F32, tag="ws1", name="ws1")
        nc.sync.dma_start(ws1_sb[:], wsum1[:])
        w1_sb = []
        with pool("mskp", 1) as mskp, pool("ptp", 4) as ptp, \
             pool("sps", 2, "PSUM") as spsp, \
             pool("avps", 1, "PSUM") as avps, \
             pool("dnps", 1, "PSUM") as dnps, \
             pool("arp", 2) as arp:
            for j in range(4):
                n_kt = 4 * j + 4
                qsl = slice(j * 512, (j + 1) * 512)
                mts = {}
                for kt in range(n_kt):
                    ksl = slice(kt * 128, (kt + 1) * 128)
                    mt = mskp.tile([128, 512], BF16, tag=f"mt{kt}",
                                   name=f"mt{j}_{kt}")
                    nc.sync.dma_start(mt[:], maskT[ksl, qsl])
                    mts[kt] = mt
                attnT = [asb.tile([128, 512], F32R, tag=f"aT{m}",
                                  name=f"aT{m}_{j}") for m in range(2)]
                rden = [asb.tile([1, 512], F32R, tag=f"rden{h}",
                                 name=f"rden{h}_{j}") for h in range(4)]
                for hp in range(2):
                    avA = avps.tile([65, 512], F32, tag="avA", name="avA")
                    avB = avps.tile([65, 512], F32, tag="avB", name="avB")
                    for kt in range(n_kt):
                        ksl = slice(kt * 128, (kt + 1) * 128)
                        sps = spsp.tile([128, 1024], F32, tag="sps", name="sps")
                        nc.tensor.matmul(
                            sps[:, 0:512], r32(kT[hp][0:64, ksl]),
                            r32(qT[hp][0:64, qsl]), start=True, stop=True,
                            tile_position=(0, 0), skip_group_check=True)
                        nc.tensor.matmul(
                            sps[:, 512:1024], r32(kT[hp][64:128, ksl]),
                            r32(qT[hp][64:128, qsl]), start=True, stop=True,
                            tile_position=(64, 0), skip_group_check=True)
                        pt = ptp.tile([128, 1024], BF16, tag="pt", name="pt")
                        nc.scalar.activation(pt[:], sps[:], AF.Exp,
                                             scale=0.125)
                        nc.vector.tensor_mul(pt[:, 0:512], pt[:, 0:512],
                                             mts[kt][:])
                        nc.vector.tensor_mul(pt[:, 512:1024],
                                             pt[:, 512:1024], mts[kt][:])
                        vv = v_sb[kt][:].rearrange("p (h c) -> p h c", h=4)
                        nc.tensor.matmul(
                            avA[:], vv[:, 2 * hp, :], pt[:, 0:512],
                            start=(kt == 0), stop=(kt == n_kt - 1),
                            skip_group_check=True)
                        nc.tensor.matmul(
                            avB[:], vv[:, 2 * hp + 1, :], pt[:, 512:1024],
                            start=(kt == 0), stop=(kt == n_kt - 1),
                            skip_group_check=True)
                    nc.vector.reciprocal(rden[2 * hp][:], avA[64:65, :])
                    nc.vector.reciprocal(rden[2 * hp + 1][:], avB[64:65, :])
                    nc.scalar.copy(attnT[hp][0:64, :], avA[0:64, :])
                    nc.scalar.copy(attnT[hp][64:128, :], avB[0:64, :])
                # normalize chunk j and project through Wo
                for m in range(2):
                    dp = dnps.tile([128, 512], F32, tag="dp", name="dp")
                    nc.tensor.matmul(dp[0:64, :], r32(ones_sb[0:1, 0:64]),
                                     rden[2 * m][:],
                                     start=True, stop=True,
                                     skip_group_check=True)
                    nc.tensor.matmul(dp[64:128, :], r32(ones_sb[0:1, 64:128]),
                                     rden[2 * m + 1][:],
                                     start=True, stop=True,
                                     skip_group_check=True)
                    nc.vector.tensor_mul(attnT[m][:], attnT[m][:], dp[:])
                for o in range(DT8):
                    osl = slice(o * 128, (o + 1) * 128)
                    wps = dnps.tile([128, 512], F32, tag="wps", name="wps")
                    for cc in range(2):
                        nc.tensor.matmul(
                            wps[:], r32(wo_sb[cc][:, osl]),
                            attnT[cc][:],
                            start=(cc == 0), stop=(cc == 1),
                            skip_group_check=True)
                    ao = arp.tile([128, 512], BF16, tag="ao", name="ao")
                    nc.scalar.copy(ao[:], wps[:])
                    nc.scalar.dma_start(rs_in[j * D + o * 128:
                                              j * D + (o + 1) * 128, :], ao[:])
        # one ReduceScatter: core g receives summed attn-out for its tokens
        nc.gpsimd.collective_compute(
            "ReduceScatter", mybir.AluOpType.add,
            replica_groups=GROUPS,
            ins=[rs_in[:]], outs=[rs_out[:]])
        s234.close()  # free qT/kT/v/attnT SBUF before phases 5-6

        # ====== phases 5+6: x2 + LN2 + token-parallel FFN (my 512 toks) ==
        with pool("w1p", 2) as w1p, pool("w2p", 2) as w2p, \
             pool("x2p", 1) as x2p, pool("ln2sb", 1) as ln2sb, \
             pool("sqp2", 2) as sqp2, pool("arl", 1) as arl, \
             pool("lnps", 2, "PSUM") as lnps, \
             pool("ffps", 2, "PSUM") as ffps, \
             pool("a1p", 1) as a1p, pool("ffo", 2) as ffop:
            # w1 streams on the SP queue (behind the attention mask loads);
            # tiles 16+ reuse slots and wait on mm1 readers.
            for hm in range(HM32):
                w1t = w1p.tile([128, DT8 * 128], BF16,
                               tag=f"w1_{hm % 8}", name=f"w1_{hm}")
                nc.sync.dma_start(
                    w1t[:], w1r[:, hm * DT8 * 128:(hm + 1) * DT8 * 128])
                w1_sb.append(w1t)
            a1 = [a1p.tile([128, TC], BF16, tag=f"a1_{hm}", name=f"a1_{hm}")
                  for hm in range(HM32)]
            # x residual loads directly into the x2 tiles (SP queue, no
            # deps); RS-dependent ar_t loads go last on SP so nothing
            # queues behind the collective wait.  w2 streams on Act queue.
            x2 = []
            for d in range(DT8):
                xt2 = x2p.tile([128, TC], F32R, tag=f"x2_{d}", name=f"x2_{d}")
                nc.scalar.dma_start(xt2[:].bitcast(F32),
                                    x_myT[d * 128:(d + 1) * 128, :])
                x2.append(xt2)
            ar_ts = []
            for d in range(DT8):
                ar_t = arl.tile([128, TC], BF16, tag=f"art{d}", name=f"art{d}")
                nc.scalar.dma_start(ar_t[:], rs_out[d * 128:(d + 1) * 128, :])
                ar_ts.append(ar_t)
            # w2 streams behind ar_t on the Act queue: tiles 4+ reuse slots
            # and wait on mm2 readers, with only outT stores behind them.
            w2_sb = []
            for om in range(DT8):
                w2t = w2p.tile([128, HM32 * 128], BF16, tag=f"w2_{om % 2}",
                               name=f"w2_{om}", bufs=2)
                nc.scalar.dma_start(
                    w2t[:], w2r[:, om * HM32 * 128:(om + 1) * HM32 * 128])
                w2_sb.append(w2t)
            for d in range(DT8):
                nc.vector.scalar_tensor_tensor(
                    x2[d][:], ar_ts[d][:], bo_sb[:, d:d + 1], x2[d][:],
                    OP.add, OP.add)
            # LN2 stats
            sum_ps = lnps.tile([128, TC], F32, tag="s2", name="s2")
            sq_ps = lnps.tile([128, TC], F32, tag="q2", name="q2")
            for d in range(DT8):
                sq = sqp2.tile([128, TC], F32R, tag="sq2", name="sq2")
                nc.scalar.activation(sq[:], x2[d][:], AF.Square)
                nc.tensor.matmul(sum_ps[:], r32(ones_sb[:]), r32(x2[d][:]),
                                 start=(d == 0), stop=(d == DT8 - 1),
                                 skip_group_check=True)
                nc.tensor.matmul(sq_ps[:], r32(ones_sb[:]), r32(sq[:]),
                                 start=(d == 0), stop=(d == DT8 - 1),
                                 skip_group_check=True)
            mu2 = ln2sb.tile([128, TC], F32, tag="mu2", name="mu2")
            nrsig2 = ln2sb.tile([128, TC], F32, tag="nrsig2", name="nrsig2")
            wrk2 = ln2sb.tile([128, TC], F32, tag="wrk2", name="wrk2")
            nc.vector.tensor_scalar_mul(mu2[:], sum_ps[:], 1.0 / D)
            nc.vector.tensor_scalar_mul(wrk2[:], sq_ps[:], 1.0 / D)
            nc.vector.tensor_tensor(nrsig2[:], mu2[:], mu2[:], OP.mult)
            nc.vector.tensor_sub(wrk2[:], wrk2[:], nrsig2[:])
            nc.vector.tensor_scalar_add(wrk2[:], wrk2[:], LN_EPS)
            nc.scalar.activation(wrk2[:], wrk2[:], AF.Sqrt)
            nc.vector.reciprocal(nrsig2[:], wrk2[:])
            murs2 = ln2sb.tile([128, TC], F32, tag="murs2", name="murs2")
            nc.vector.tensor_tensor(murs2[:], mu2[:], nrsig2[:], OP.mult)
            # x2s = x2 * rsig2 (bf16); correction folded via stt + gelu
            x2s = []
            for d in range(DT8):
                xs_ = sqp2.tile([128, TC], BF16, tag=f"x2s_{d}",
                                name=f"x2s_{d}", bufs=1)
                nc.vector.tensor_tensor(xs_[:], x2[d][:], nrsig2[:], OP.mult)
                x2s.append(xs_)
            for hm in range(HM32):
                ap_ = ffps.tile([128, TC], F32, tag="a1ps", name="a1ps")
                for d in range(DT8):
                    nc.tensor.matmul(
                        ap_[:], w1_sb[hm][:, d * 128:(d + 1) * 128],
                        x2s[d][:], start=(d == 0), stop=(d == DT8 - 1),
                        skip_group_check=True)
                a1n = sqp2.tile([128, TC], F32, tag="a1n", name="a1n")
                nc.vector.scalar_tensor_tensor(
                    a1n[:], murs2[:], ws1_sb[:, hm:hm + 1], ap_[:],
                    OP.mult, OP.subtract)
                nc.scalar.activation(a1[hm][:], a1n[:], AF.Gelu,
                                     bias=b1_sb[:, hm:hm + 1], scale=-1.0)
            # mm2 + residual + b2 -> final out
            for om in range(DT8):
                osl = slice(om * 128, (om + 1) * 128)
                fo = ffop.tile([128, TC], F32, tag="fo", name="fo")
                fp_ = ffps.tile([128, TC], F32, tag="ffps", name="ffps")
                for hm in range(HM32):
                    nc.tensor.matmul(
                        fp_[:], w2_sb[om][:, hm * 128:(hm + 1) * 128],
                        a1[hm][:], start=(hm == 0), stop=(hm == HM32 - 1),
                        skip_group_check=True)
                nc.vector.scalar_tensor_tensor(
                    fo[:], fp_[:], b2_sb[:, om:om + 1], x2[om][:],
                    OP.add, OP.add)
                nc.scalar.dma_start(outT[osl, :], fo[:])
    nc.compile()
    return nc


def host_prep(inputs):
    """Build per-core input maps from the full problem inputs."""
    x = np.asarray(inputs["x"], np.float32)
    mask = np.asarray(inputs["mask"])
    ln1_g = np.asarray(inputs["ln1_g"], np.float32)
    ln1_b = np.asarray(inputs["ln1_b"], np.float32)
    ln2_g = np.asarray(inputs["ln2_g"], np.float32)
    ln2_b = np.asarray(inputs["ln2_b"], np.float32)
    Wq = np.asarray(inputs["Wq"], np.float32)
    Wk = np.asarray(inputs["Wk"], np.float32)
    Wv = np.asarray(inputs["Wv"], np.float32)
    Wo = np.asarray(inputs["Wo"], np.float32)
    bo = np.asarray(inputs["bo"], np.float32)
    W1 = np.asarray(inputs["W1"], np.float32)
    b1 = np.asarray(inputs["b1"], np.float32)
    W2 = np.asarray(inputs["W2"], np.float32)
    b2 = np.asarray(inputs["b2"], np.float32)

    maskT = np.ascontiguousarray(mask.T).astype(np.float32).astype(NPBF16)
    Wq_f = ln1_g[:, None] * Wq
    Wk_f = ln1_g[:, None] * Wk
    Wv_f = ln1_g[:, None] * Wv
    W1_f = ln2_g[:, None] * W1
    # w1r block (hm*8+d) = W1_f[d-tile, hm-tile]; w2r block (om*32+hm)
    w1r = np.ascontiguousarray(
        W1_f.reshape(DT8, 128, HM32, 128).transpose(1, 2, 0, 3)
        .reshape(128, HM32 * DT8 * 128)).astype(NPBF16)
    w2r = np.ascontiguousarray(
        W2.reshape(HM32, 128, DT8, 128).transpose(1, 2, 0, 3)
        .reshape(128, DT8 * HM32 * 128)).astype(NPBF16)
    b1_col = (ln2_b @ W1_f + b1).reshape(HM32, 128).T.copy()
    wsum1 = W1_f.sum(0).reshape(HM32, 128).T.copy()
    bo_col = bo.reshape(DT8, 128).T.copy()
    b2_col = b2.reshape(DT8, 128).T.copy()
    xT = [np.ascontiguousarray(x[b].T) for b in range(B)]
    in_maps = []
    for c in range(NCORES):
        b, g = divmod(c, 4)
        cs = slice(g * C, (g + 1) * C)
        wq_s = np.ascontiguousarray(Wq_f[:, cs])
        wk_s = np.ascontiguousarray(Wk_f[:, cs])
        wv_s = np.ascontiguousarray(Wv_f[:, cs])
        m = {
            "xT": xT[b],
            "x_myT": np.ascontiguousarray(xT[b][:, g * TC:(g + 1) * TC]),
            "maskT": maskT,
            "wq": wq_s, "wk": wk_s, "wv": wv_s,
            "wo": np.ascontiguousarray(Wo[cs, :]),
            "w1r": w1r, "w2r": w2r,
            "wsq": wq_s.sum(0).reshape(2, 128).T.copy(),
            "wsk": wk_s.sum(0).reshape(2, 128).T.copy(),
            "wsv_bc": np.broadcast_to(wv_s.sum(0), (128, C)).copy(),
            "bq": (ln1_b @ Wq[:, cs]).reshape(2, 128).T.copy(),
            "bk": (ln1_b @ Wk[:, cs]).reshape(2, 128).T.copy(),
            "bv_bc": np.broadcast_to(ln1_b @ Wv[:, cs], (128, C)).copy(),
            "bo_col": bo_col,
            "b1_col": b1_col,
            "wsum1": wsum1,
            "b2_col": b2_col,
        }
        in_maps.append(m)
    return in_maps, b2


def host_assemble(out_maps, b2):
    out = np.empty((B, T, D), np.float32)
    for c in range(NCORES):
        b, g = divmod(c, 4)
        out[b, g * TC:(g + 1) * TC, :] = out_maps[c]["outT"].T
    return out


# ======================================================================
# Harness entry point
# ======================================================================
_NC_CACHE = {}


def _get_nc():
    if "nc" not in _NC_CACHE:
        _NC_CACHE["nc"] = build_nc()
    return _NC_CACHE["nc"]


def kernel(**inputs):
    """Full-input / full-output BigBird block on 8 NeuronCores."""
    from concourse.bass_utils import run_bass_kernel_spmd
    nc = _get_nc()
    in_maps, b2 = host_prep(inputs)
    res = run_bass_kernel_spmd(nc, in_maps, list(range(NCORES)))
    return host_assemble(res.results, b2)

